# revision 45
# baseline (speedup 1.0000x reference)
"""Trainium2 Bass kernel for CAM (channel attention module).

Reference computation (per batch b):
    q = x_low[b]  as [C, N]   (C=512, N=64*64=4096)
    k = x_high[b] as [C, N]
    E = q @ k.T                              # [C, C]
    att = softmax(rowmax(E) - E, axis=-1)    # == exp(rowmin(E) - E) / Z
    out = gamma * (att @ k) + x_low[b]

Sharding: data-parallel over batch. 16 batches / 8 cores = 2 per core.
gamma is replicated (pre-broadcast on host to [128,1]).

Inputs are cast to fp16 on the host (numerically validated: max rel err
~8.9e-3 vs the fp32 reference, under the 2e-2 gate), halving HBM load
traffic. mm1 contracts over N, which needs n-major operands: k^T is fed
as an extra host-prepared input (xhT; layout prep on host, +4 MiB/batch
of DMA), while q^T is produced on-chip with fp16 PE transposes
(1 cycle/row) evicted PSUM->SBUF on DVE/ACT. Matmuls run fp16 in, fp32
PSUM accumulation. The residual add rides the DVE PSUM->SBUF eviction
of mm2. Load order puts q-naturals and the first k^T group first so the
PE starts ~7us sooner; k-naturals (only needed by mm2) come last.

The two batches are software-pipelined on the PE queue: batch b+1's
transposes+mm1 are interleaved 3:1 with batch b's mm2; per-chunk softmax
steps alternate with b+1 pipeline steps so the chains share DVE/ACT;
b+1's mm1 lags its transposes by 4 chunks (t_ahead) because its E-bank
writes WAR on b's banks, which free only as b's exp chain reads them;
and the sync DMA ring is ordered by first-use time (b+1's q-s0/kT head
is emitted before b's mm2-only k naturals, since the FIFO ring would
otherwise delay b+1's first transpose by ~4 us). Measured ~159.6 us on
hardware (baseline f32r PE-transpose version: 229.8 us); PE busy ~133 us
with <1 us of mid-kernel gaps, DMA ~123 us, plus ~8 us runtime start and
~9 us end barrier.

Dead ends measured: DMA x-bar transpose emits 256 B descriptors (~7
GB/s/engine, 2.3x slower than plain DMA per byte) and can wedge the
device when interleaved with loads on the sync queue; fp8e4m3 mm2 gives
3.9e-2 max rel err (fails); bf16 mm1 gives 1e-1 (fails); interleaving
the two batches' mm2 streams deadlocks on attT pool WAR or loses time
to PSUM rotation stalls.
"""

import sys

sys.path.insert(0, "/opt/trn_rl_repo")

import numpy as np

B, C, H, W = 16, 512, 64, 64
N = H * W               # 4096
N_CORES = 8
B_LOC = B // N_CORES    # 2 batches per core
P = 128                 # partitions
CP = C // P             # 4 channel chunks
ST = 1024               # load sub-tile free size (fp16 words)
NS = N // ST            # 4 sub-tiles per (tensor, cc)
NN = N // P             # 32 n chunks of 128
FB = 512                # free-dim block (psum bank) for mm2 output
NB = N // FB            # 8 n blocks of 512

_CACHE = {}


def _build(reps=0, variant="full"):
    import contextlib
    import concourse.bacc as bacc
    import concourse.tile as tile
    import concourse.mybir as mybir
    from concourse.masks import make_identity

    f32 = mybir.dt.float32
    f16 = mybir.dt.float16

    nc = bacc.Bacc("TRN2", target_bir_lowering=False, debug=False)

    xh = nc.dram_tensor("xh", [B_LOC, C, N], f16, kind="ExternalInput")
    xhT = nc.dram_tensor("xhT", [B_LOC, N, C], f16, kind="ExternalInput")
    xl = nc.dram_tensor("xl", [B_LOC, C, N], f16, kind="ExternalInput")
    gm = nc.dram_tensor("gm", [P, 1], f32, kind="ExternalInput")
    out = nc.dram_tensor("out", [B_LOC, C, N], f32, kind="ExternalOutput")

    KG = 4               # mm1 n-chunks per kT load group
    NG = NN // KG        # 8 kT load groups per batch

    with tile.TileContext(nc) as tc:
        with (
            tc.tile_pool(name="const", bufs=1) as const_pool,
            tc.tile_pool(name="kn", bufs=NS * CP + 6) as kn_pool,
            tc.tile_pool(name="qn", bufs=2 * NS * CP) as qn_pool,
            tc.tile_pool(name="kT", bufs=NG + 1) as kT_pool,
            tc.tile_pool(name="qT", bufs=6) as qT_pool,
            tc.tile_pool(name="att", bufs=CP) as att_pool,
            tc.tile_pool(name="attT", bufs=CP) as attT_pool,
            tc.tile_pool(name="osb", bufs=4) as out_pool,
            tc.tile_pool(name="small", bufs=32) as small_pool,
            tc.tile_pool(name="psE", bufs=CP, space="PSUM") as psE_pool,
            tc.tile_pool(name="psT", bufs=2, space="PSUM") as psT_pool,
            tc.tile_pool(name="psA", bufs=2, space="PSUM") as psA_pool,
        ):
            ident_f = const_pool.tile([P, P], f32)
            make_identity(nc, ident_f[:])
            identh = const_pool.tile([P, P], f16)
            nc.vector.tensor_copy(identh[:], ident_f[:])
            gsb = const_pool.tile([P, 1], f32)
            nc.sync.dma_start(gsb[:], gm.ap())

            # warm-up: dummy transposes while the first loads are in flight.
            # PE_HAM starts throttled (K/N clock gate) and releases only after
            # ~4us of sustained activity; burning that ramp on the identity
            # means the first real transposes/matmuls run at full clock.
            warm = psT_pool.tile([P, FB], f16, tag="wp")
            for w in range(56):
                nc.tensor.transpose(warm[:, (w % CP) * P:(w % CP + 1) * P],
                                    identh[:], identh[:])

            class BatchState:
                pass

            def make_state(b):
                st = BatchState()
                st.b = b
                st.KN = [[None] * NS for _ in range(CP)]
                st.QN = [[None] * NS for _ in range(CP)]
                st.KT = [None] * NG
                st.QT = [None] * NN
                st.E = None
                st.att = []
                st.attT = []
                return st

            def emit_loads(st, s_lo=0, s_hi=NS):
                # q naturals + kT groups for sub-tiles [s_lo, s_hi): the mm1
                # feed. The transpose of chunk 0 only needs q-s0, so callers
                # emit batch b+1's head before batch b's mm2-only k naturals.
                b = st.b
                xhTb = xhT.ap()[b].rearrange("(g j p) c -> g p j c", p=P, j=KG)
                for s in range(s_lo, s_hi):
                    ssl = slice(s * ST, (s + 1) * ST)
                    for cc in range(CP):
                        csl = slice(cc * P, (cc + 1) * P)
                        qt = qn_pool.tile([P, ST], f16, tag="qn", name=f"qn{b}_{cc}_{s}")
                        nc.sync.dma_start(qt[:], xl.ap()[b, csl, ssl])
                        st.QN[cc][s] = qt
                    for g in range(2 * s, 2 * s + 2):
                        kT4 = kT_pool.tile([P, KG, C], f16, tag="kT",
                                           name=f"kT{b}_{g}")
                        nc.sync.dma_start(kT4[:], xhTb[g])
                        st.KT[g] = kT4

            def emit_loads_kn(st):
                b = st.b
                for s in range(NS):
                    ssl = slice(s * ST, (s + 1) * ST)
                    for cc in range(CP):
                        csl = slice(cc * P, (cc + 1) * P)
                        kt = kn_pool.tile([P, ST], f16, tag="kn", name=f"kn{b}_{cc}_{s}")
                        nc.sync.dma_start(kt[:], xh.ap()[b, csl, ssl])
                        st.KN[cc][s] = kt

            def nat_blk(tiles, cc, lo, width):
                s = lo // ST
                o = lo - s * ST
                return tiles[cc][s][:, o:o + width]

            def gen_tmm1(st, t_ahead=1):
                """PE transposes of n-chunk nn + mm1 lagging t_ahead chunks.
                A larger t_ahead front-loads softmax-independent transposes:
                this batch's first mm1 writes WAR on the previous batch's E
                banks, which free only as its exp chain reads them."""
                b = st.b
                st.E = [psE_pool.tile([P, FB], f32, tag="E", name=f"E{b}_{i}")
                        for i in range(CP)]

                def emit_T(nn):
                    qtp = psT_pool.tile([P, FB], f16, tag="wp")
                    for cc in range(CP):
                        csl = slice(cc * P, (cc + 1) * P)
                        nc.tensor.transpose(
                            qtp[:, csl], nat_blk(st.QN, cc, nn * P, P), identh[:])
                    qTn = qT_pool.tile([P, FB], f16, tag="qT", name=f"qT{b}_{nn}")
                    if nn % 2 == 0:
                        nc.vector.tensor_copy(qTn[:], qtp[:])
                    else:
                        nc.scalar.copy(qTn[:], qtp[:])
                    st.QT[nn] = qTn

                def emit_mm1(nn):
                    for ic in range(CP):
                        nc.tensor.matmul(
                            st.E[ic][:],
                            st.QT[nn][:, ic * P:(ic + 1) * P],
                            st.KT[nn // KG][:, nn % KG, :],
                            start=(nn == 0),
                            stop=(nn == NN - 1),
                        )

                for nn in range(NN):
                    emit_T(nn)
                    if nn >= t_ahead:
                        emit_mm1(nn - t_ahead)
                    yield
                for nn in range(NN - t_ahead, NN):
                    emit_mm1(nn)
                    yield

            def gen_softmax(st):
                # DVE/ACT only -- no PE instructions; one yield per channel
                # chunk so callers can interleave with PE-feeding copies
                for ic in range(CP):
                    m = small_pool.tile([P, 1], f32, tag="m")
                    nc.vector.tensor_reduce(
                        m[:], st.E[ic][:], axis=mybir.AxisListType.X,
                        op=mybir.AluOpType.min,
                    )
                    a = att_pool.tile([P, FB], f16, tag="att")
                    z = small_pool.tile([P, 1], f32, tag="z")
                    nc.scalar.activation(
                        a[:], st.E[ic][:], mybir.ActivationFunctionType.Exp,
                        bias=m[:], scale=-1.0, accum_out=z[:],
                    )
                    zinv = small_pool.tile([P, 1], f32, tag="zi")
                    nc.vector.reciprocal(zinv[:], z[:])
                    asc = small_pool.tile([P, 1], f32, tag="as")
                    nc.vector.tensor_mul(asc[:], zinv[:], gsb[:])
                    nc.vector.tensor_scalar_mul(a[:], a[:], asc[:])
                    st.att.append(a)
                    yield

            def emit_attT(st):
                for jc in range(CP):
                    atp = psT_pool.tile([P, FB], f16, tag="wp")
                    jsl = slice(jc * P, (jc + 1) * P)
                    for ic in range(CP):
                        nc.tensor.transpose(
                            atp[:, ic * P:(ic + 1) * P], st.att[ic][:, jsl], identh[:]
                        )
                    aT = attT_pool.tile([P, FB], f16, tag="attT")
                    if jc % 2 == 0:
                        nc.vector.tensor_copy(aT[:], atp[:])
                    else:
                        nc.scalar.copy(aT[:], atp[:])
                    st.attT.append(aT)

            def gen_mm2(st, split_last=0):
                """32 steps: one (nb, ic) accumulation + residual; store per nb.
                The last `split_last` n-blocks store per-ic so the drain tail
                after the final matmul is a 256 KiB transfer, not 1 MiB."""
                b = st.b
                dst = out.ap()[b].rearrange("(i p) n -> p i n", p=P)
                for nb in range(NB):
                    fine = nb >= NB - split_last
                    osb = out_pool.tile([P, CP, FB], f32, tag="osb")
                    for ic in range(CP):
                        isl = slice(ic * P, (ic + 1) * P)
                        acc = psA_pool.tile([P, FB], f32, tag="acc",
                                            name=f"acc{b}_{nb}_{ic}")
                        for jc in range(CP):
                            nc.tensor.matmul(
                                acc[:],
                                st.attT[jc][:, isl],
                                nat_blk(st.KN, jc, nb * FB, FB),
                                start=(jc == 0),
                                stop=(jc == CP - 1),
                            )
                        nc.vector.tensor_add(
                            osb[:, ic, :], acc[:], nat_blk(st.QN, ic, nb * FB, FB))
                        if fine:
                            # HWDGE for the drain-critical tail stores: ~0.6us
                            # first-byte vs SWDGE's ~1us Q7 descriptor-gen
                            nc.sync.dma_start(
                                dst[:, ic:ic + 1, nb * FB:(nb + 1) * FB],
                                osb[:, ic:ic + 1, :])
                        yield
                    if not fine:
                        nc.gpsimd.dma_start(dst[:, :, nb * FB:(nb + 1) * FB], osb[:])

            def emit_dma_variant(st):
                b = st.b
                dst = out.ap()[b].rearrange("(i p) n -> p i n", p=P)
                for nb in range(NB):
                    osb = out_pool.tile([P, CP, FB], f32, tag="osb")
                    for ic in range(CP):
                        src = (nat_blk(st.QN, ic, nb * FB, FB) if nb % 2 == 0
                               else nat_blk(st.KN, ic, nb * FB, FB))
                        nc.vector.tensor_copy(osb[:, ic, :], src)
                    nc.gpsimd.dma_start(dst[:, :, nb * FB:(nb + 1) * FB], osb[:])

            def drain(g, n=None):
                i = 0
                for _ in g:
                    i += 1
                    if n is not None and i >= n:
                        return True
                return False

            rep_ctx = tc.For_i(0, reps, 1) if reps else contextlib.nullcontext()
            with rep_ctx:
                states = [make_state(b) for b in range(B_LOC)]
                # sync DMA ring is FIFO: order loads by first-use time.
                # b1's T(0) needs its q-s0 before b0's mm2 needs b0's kn.
                emit_loads(states[0])
                emit_loads(states[1], 0, 1)
                emit_loads_kn(states[0])
                emit_loads(states[1], 1, NS)
                emit_loads_kn(states[1])

                if variant == "dma":
                    for st in states:
                        emit_dma_variant(st)
                else:
                    # software-pipelined schedule over the two batches
                    s0, s1 = states
                    g0 = gen_tmm1(s0)
                    drain(g0)                 # batch0 transposes + mm1
                    g1 = gen_tmm1(s1, t_ahead=4)
                    sm0 = gen_softmax(s0)
                    for _ in range(CP):
                        # alternate per-chunk softmax with b1 pipeline steps so
                        # the softmax chain and b1's PSUM->SBUF copies share
                        # DVE/ACT evenly instead of queue-blocking each other
                        drain(sm0, 1)
                        drain(g1, 1)
                    drain(g1, 2)
                    emit_attT(s0)
                    m0 = gen_mm2(s0)
                    t = 4
                    more = True
                    while more:
                        more = drain(g1, 1)
                        t += 1
                        if t % 4 != 0:
                            drain(m0, 1)
                    sm1 = gen_softmax(s1)
                    more = True
                    for _ in range(CP):
                        # same interleave at the b1 boundary: softmax chunks
                        # alternate with b0's leftover mm2 (PE) + adds (DVE)
                        drain(sm1, 1)
                        more = drain(m0, 1)
                    if more:
                        drain(m0)             # PE: leftover mm2 fills the gap
                    emit_attT(s1)
                    drain(gen_mm2(s1, split_last=3))

    nc.compile()
    return nc


def _get_module():
    if "nc" not in _CACHE:
        _CACHE["nc"] = _build()
    return _CACHE["nc"]


def prepare_in_maps(x_high, x_low, gamma):
    x_high = np.asarray(x_high)
    x_low = np.asarray(x_low)
    gamma = np.asarray(gamma, dtype=np.float32).reshape(-1)

    xh3 = x_high.reshape(B, C, N).astype(np.float16)
    xl3 = x_low.reshape(B, C, N).astype(np.float16)
    xh3T = np.ascontiguousarray(xh3.transpose(0, 2, 1))
    gm = np.full((P, 1), gamma[0], dtype=np.float32)

    in_maps = []
    for i in range(N_CORES):
        sl = slice(i * B_LOC, (i + 1) * B_LOC)
        in_maps.append({
            "xh": np.ascontiguousarray(xh3[sl]),
            "xhT": np.ascontiguousarray(xh3T[sl]),
            "xl": np.ascontiguousarray(xl3[sl]),
            "gm": gm,
        })
    return in_maps


def kernel(x_high, x_low, gamma):
    from concourse.bass_utils import run_bass_kernel_spmd

    nc = _get_module()
    in_maps = prepare_in_maps(x_high, x_low, gamma)
    res = run_bass_kernel_spmd(nc, in_maps, list(range(N_CORES)))
    out = np.concatenate([res.results[i]["out"] for i in range(N_CORES)], axis=0)
    return out.reshape(B, C, H, W)


# revision 47
# speedup vs baseline: 1.1566x; 1.1566x over previous
"""Trainium2 Bass kernel for CAM (channel attention module).

Reference computation (per batch b):
    q = x_low[b]  as [C, N]   (C=512, N=64*64=4096)
    k = x_high[b] as [C, N]
    E = q @ k.T                              # [C, C]
    att = softmax(rowmax(E) - E, axis=-1)    # == exp(rowmin(E) - E) / Z
    out = gamma * (att @ k) + x_low[b]

Sharding: data-parallel over batch. 16 batches / 8 cores = 2 per core.
gamma is replicated (pre-broadcast on host to [128,1]).

Inputs are cast to fp16 on the host (numerically validated: max rel err
~8.9e-3 vs the fp32 reference, under the 2e-2 gate), halving HBM load
traffic. mm1 contracts over N, which needs n-major operands: k^T is fed
as an extra host-prepared input (xhT; layout prep on host, +4 MiB/batch
of DMA), while q^T is produced on-chip with fp16 PE transposes
(1 cycle/row) evicted PSUM->SBUF on DVE/ACT. Matmuls run fp16 in, fp32
PSUM accumulation. The residual add rides the DVE PSUM->SBUF eviction
of mm2. Load order puts q-naturals and the first k^T group first so the
PE starts ~7us sooner; k-naturals (only needed by mm2) come last.

The two batches are software-pipelined on the PE queue: batch b+1's
transposes+mm1 are interleaved 3:1 with batch b's mm2; per-chunk softmax
steps alternate with b+1 pipeline steps so the chains share DVE/ACT;
b+1's mm1 lags its transposes by 4 chunks (t_ahead) because its E-bank
writes WAR on b's banks, which free only as b's exp chain reads them;
and the sync DMA ring is ordered by first-use time (b+1's q-s0/kT head
is emitted before b's mm2-only k naturals, since the FIFO ring would
otherwise delay b+1's first transpose by ~4 us). Measured ~159.6 us on
hardware (baseline f32r PE-transpose version: 229.8 us); PE busy ~133 us
with <1 us of mid-kernel gaps, DMA ~123 us, plus ~8 us runtime start and
~9 us end barrier.

Dead ends measured: DMA x-bar transpose emits 256 B descriptors (~7
GB/s/engine, 2.3x slower than plain DMA per byte) and can wedge the
device when interleaved with loads on the sync queue; fp8e4m3 mm2 gives
3.9e-2 max rel err (fails); bf16 mm1 gives 1e-1 (fails); interleaving
the two batches' mm2 streams deadlocks on attT pool WAR or loses time
to PSUM rotation stalls.
"""

import sys

sys.path.insert(0, "/opt/trn_rl_repo")

import numpy as np

B, C, H, W = 16, 512, 64, 64
N = H * W               # 4096
N_CORES = 8
B_LOC = B // N_CORES    # 2 batches per core
P = 128                 # partitions
CP = C // P             # 4 channel chunks
ST = 1024               # load sub-tile free size (fp16 words)
NS = N // ST            # 4 sub-tiles per (tensor, cc)
NN = N // P             # 32 n chunks of 128
FB = 512                # free-dim block (psum bank) for mm2 output
NB = N // FB            # 8 n blocks of 512

_CACHE = {}


def _build(reps=0, variant="full"):
    import contextlib
    import concourse.bacc as bacc
    import concourse.tile as tile
    import concourse.mybir as mybir
    from concourse.masks import make_identity

    f32 = mybir.dt.float32
    f16 = mybir.dt.float16

    nc = bacc.Bacc("TRN2", target_bir_lowering=False, debug=False)

    xh = nc.dram_tensor("xh", [B_LOC, C, N], f16, kind="ExternalInput")
    xhT = nc.dram_tensor("xhT", [B_LOC, N, C], f16, kind="ExternalInput")
    xl = nc.dram_tensor("xl", [B_LOC, C, N], f16, kind="ExternalInput")
    gm = nc.dram_tensor("gm", [P, 1], f32, kind="ExternalInput")
    out = nc.dram_tensor("out", [B_LOC, C, N], f32, kind="ExternalOutput")

    KG = 4               # mm1 n-chunks per kT load group
    NG = NN // KG        # 8 kT load groups per batch

    with tile.TileContext(nc) as tc:
        with (
            tc.tile_pool(name="const", bufs=1) as const_pool,
            tc.tile_pool(name="kn", bufs=NS * CP + 6) as kn_pool,
            tc.tile_pool(name="qn", bufs=2 * NS * CP) as qn_pool,
            tc.tile_pool(name="kT", bufs=NG + 1) as kT_pool,
            tc.tile_pool(name="qT", bufs=6) as qT_pool,
            tc.tile_pool(name="att", bufs=CP) as att_pool,
            tc.tile_pool(name="attT", bufs=CP) as attT_pool,
            tc.tile_pool(name="osb", bufs=4) as out_pool,
            tc.tile_pool(name="small", bufs=32) as small_pool,
            tc.tile_pool(name="psE", bufs=CP, space="PSUM") as psE_pool,
            tc.tile_pool(name="psT", bufs=2, space="PSUM") as psT_pool,
            tc.tile_pool(name="psA", bufs=2, space="PSUM") as psA_pool,
        ):
            ident_f = const_pool.tile([P, P], f32)
            make_identity(nc, ident_f[:])
            identh = const_pool.tile([P, P], f16)
            nc.vector.tensor_copy(identh[:], ident_f[:])
            gsb = const_pool.tile([P, 1], f32)
            nc.sync.dma_start(gsb[:], gm.ap())

            # warm-up: dummy transposes while the first loads are in flight.
            # PE_HAM starts throttled (K/N clock gate) and releases only after
            # ~4us of sustained activity; burning that ramp on the identity
            # means the first real transposes/matmuls run at full clock.
            warm = psT_pool.tile([P, FB], f16, tag="wp")
            for w in range(40):
                nc.tensor.transpose(warm[:, (w % CP) * P:(w % CP + 1) * P],
                                    identh[:], identh[:])

            class BatchState:
                pass

            def make_state(b):
                st = BatchState()
                st.b = b
                st.KN = [[None] * NS for _ in range(CP)]
                st.QN = [[None] * NS for _ in range(CP)]
                st.KT = [None] * NG
                st.QT = [None] * NN
                st.E = None
                st.att = []
                st.attT = []
                return st

            def emit_loads(st, s_lo=0, s_hi=NS):
                # q naturals + kT groups for sub-tiles [s_lo, s_hi): the mm1
                # feed. The transpose of chunk 0 only needs q-s0, so callers
                # emit batch b+1's head before batch b's mm2-only k naturals.
                b = st.b
                xhTb = xhT.ap()[b].rearrange("(g j p) c -> g p j c", p=P, j=KG)
                for s in range(s_lo, s_hi):
                    ssl = slice(s * ST, (s + 1) * ST)
                    for cc in range(CP):
                        csl = slice(cc * P, (cc + 1) * P)
                        qt = qn_pool.tile([P, ST], f16, tag="qn", name=f"qn{b}_{cc}_{s}")
                        nc.sync.dma_start(qt[:], xl.ap()[b, csl, ssl])
                        st.QN[cc][s] = qt
                    for g in range(2 * s, 2 * s + 2):
                        kT4 = kT_pool.tile([P, KG, C], f16, tag="kT",
                                           name=f"kT{b}_{g}")
                        nc.sync.dma_start(kT4[:], xhTb[g])
                        st.KT[g] = kT4

            def emit_loads_kn(st):
                b = st.b
                for s in range(NS):
                    ssl = slice(s * ST, (s + 1) * ST)
                    for cc in range(CP):
                        csl = slice(cc * P, (cc + 1) * P)
                        kt = kn_pool.tile([P, ST], f16, tag="kn", name=f"kn{b}_{cc}_{s}")
                        nc.sync.dma_start(kt[:], xh.ap()[b, csl, ssl])
                        st.KN[cc][s] = kt

            def nat_blk(tiles, cc, lo, width):
                s = lo // ST
                o = lo - s * ST
                return tiles[cc][s][:, o:o + width]

            def gen_tmm1(st, t_ahead=1):
                """PE transposes of n-chunk nn + mm1 lagging t_ahead chunks.
                A larger t_ahead front-loads softmax-independent transposes:
                this batch's first mm1 writes WAR on the previous batch's E
                banks, which free only as its exp chain reads them."""
                b = st.b
                st.E = [psE_pool.tile([P, FB], f32, tag="E", name=f"E{b}_{i}")
                        for i in range(CP)]

                def emit_T(nn):
                    qtp = psT_pool.tile([P, FB], f16, tag="wp")
                    for cc in range(CP):
                        csl = slice(cc * P, (cc + 1) * P)
                        nc.tensor.transpose(
                            qtp[:, csl], nat_blk(st.QN, cc, nn * P, P), identh[:])
                    qTn = qT_pool.tile([P, FB], f16, tag="qT", name=f"qT{b}_{nn}")
                    if nn % 2 == 0:
                        nc.vector.tensor_copy(qTn[:], qtp[:])
                    else:
                        nc.scalar.copy(qTn[:], qtp[:])
                    st.QT[nn] = qTn

                def emit_mm1(nn):
                    for ic in range(CP):
                        nc.tensor.matmul(
                            st.E[ic][:],
                            st.QT[nn][:, ic * P:(ic + 1) * P],
                            st.KT[nn // KG][:, nn % KG, :],
                            start=(nn == 0),
                            stop=(nn == NN - 1),
                        )

                for nn in range(NN):
                    emit_T(nn)
                    if nn >= t_ahead:
                        emit_mm1(nn - t_ahead)
                    yield
                for nn in range(NN - t_ahead, NN):
                    emit_mm1(nn)
                    yield

            def gen_softmax(st):
                # DVE/ACT only -- no PE instructions; one yield per channel
                # chunk so callers can interleave with PE-feeding copies
                for ic in range(CP):
                    m = small_pool.tile([P, 1], f32, tag="m")
                    nc.vector.tensor_reduce(
                        m[:], st.E[ic][:], axis=mybir.AxisListType.X,
                        op=mybir.AluOpType.min,
                    )
                    a = att_pool.tile([P, FB], f16, tag="att")
                    z = small_pool.tile([P, 1], f32, tag="z")
                    nc.scalar.activation(
                        a[:], st.E[ic][:], mybir.ActivationFunctionType.Exp,
                        bias=m[:], scale=-1.0, accum_out=z[:],
                    )
                    zinv = small_pool.tile([P, 1], f32, tag="zi")
                    nc.vector.reciprocal(zinv[:], z[:])
                    asc = small_pool.tile([P, 1], f32, tag="as")
                    nc.vector.tensor_mul(asc[:], zinv[:], gsb[:])
                    nc.vector.tensor_scalar_mul(a[:], a[:], asc[:])
                    st.att.append(a)
                    yield

            def emit_attT(st):
                for jc in range(CP):
                    atp = psT_pool.tile([P, FB], f16, tag="wp")
                    jsl = slice(jc * P, (jc + 1) * P)
                    for ic in range(CP):
                        nc.tensor.transpose(
                            atp[:, ic * P:(ic + 1) * P], st.att[ic][:, jsl], identh[:]
                        )
                    aT = attT_pool.tile([P, FB], f16, tag="attT")
                    if jc % 2 == 0:
                        nc.vector.tensor_copy(aT[:], atp[:])
                    else:
                        nc.scalar.copy(aT[:], atp[:])
                    st.attT.append(aT)

            def gen_mm2(st, split_last=0):
                """32 steps: one (nb, ic) accumulation + residual; store per nb.
                The last `split_last` n-blocks store per-ic so the drain tail
                after the final matmul is a 256 KiB transfer, not 1 MiB."""
                b = st.b
                dst = out.ap()[b].rearrange("(i p) n -> p i n", p=P)
                for nb in range(NB):
                    fine = nb >= NB - split_last
                    osb = out_pool.tile([P, CP, FB], f32, tag="osb")
                    for ic in range(CP):
                        isl = slice(ic * P, (ic + 1) * P)
                        acc = psA_pool.tile([P, FB], f32, tag="acc",
                                            name=f"acc{b}_{nb}_{ic}")
                        for jc in range(CP):
                            nc.tensor.matmul(
                                acc[:],
                                st.attT[jc][:, isl],
                                nat_blk(st.KN, jc, nb * FB, FB),
                                start=(jc == 0),
                                stop=(jc == CP - 1),
                            )
                        nc.vector.tensor_add(
                            osb[:, ic, :], acc[:], nat_blk(st.QN, ic, nb * FB, FB))
                        if fine:
                            nc.gpsimd.dma_start(
                                dst[:, ic:ic + 1, nb * FB:(nb + 1) * FB],
                                osb[:, ic:ic + 1, :])
                        yield
                    if not fine:
                        nc.gpsimd.dma_start(dst[:, :, nb * FB:(nb + 1) * FB], osb[:])

            def emit_dma_variant(st):
                b = st.b
                dst = out.ap()[b].rearrange("(i p) n -> p i n", p=P)
                for nb in range(NB):
                    osb = out_pool.tile([P, CP, FB], f32, tag="osb")
                    for ic in range(CP):
                        src = (nat_blk(st.QN, ic, nb * FB, FB) if nb % 2 == 0
                               else nat_blk(st.KN, ic, nb * FB, FB))
                        nc.vector.tensor_copy(osb[:, ic, :], src)
                    nc.gpsimd.dma_start(dst[:, :, nb * FB:(nb + 1) * FB], osb[:])

            def drain(g, n=None):
                i = 0
                for _ in g:
                    i += 1
                    if n is not None and i >= n:
                        return True
                return False

            rep_ctx = tc.For_i(0, reps, 1) if reps else contextlib.nullcontext()
            with rep_ctx:
                states = [make_state(b) for b in range(B_LOC)]
                # sync DMA ring is FIFO: order loads by first-use time.
                # b1's T(0) needs its q-s0 before b0's mm2 needs b0's kn.
                emit_loads(states[0])
                emit_loads(states[1], 0, 1)
                emit_loads_kn(states[0])
                emit_loads(states[1], 1, NS)
                emit_loads_kn(states[1])

                if variant == "dma":
                    for st in states:
                        emit_dma_variant(st)
                else:
                    # software-pipelined schedule over the two batches
                    s0, s1 = states
                    g0 = gen_tmm1(s0)
                    drain(g0)                 # batch0 transposes + mm1
                    g1 = gen_tmm1(s1, t_ahead=4)
                    sm0 = gen_softmax(s0)
                    for _ in range(CP):
                        # alternate per-chunk softmax with b1 pipeline steps so
                        # the softmax chain and b1's PSUM->SBUF copies share
                        # DVE/ACT evenly instead of queue-blocking each other
                        drain(sm0, 1)
                        drain(g1, 1)
                    drain(g1, 2)
                    emit_attT(s0)
                    m0 = gen_mm2(s0)
                    t = 4
                    more = True
                    while more:
                        more = drain(g1, 1)
                        t += 1
                        if t % 4 != 0:
                            drain(m0, 1)
                    sm1 = gen_softmax(s1)
                    more = True
                    for _ in range(CP):
                        # same interleave at the b1 boundary: softmax chunks
                        # alternate with b0's leftover mm2 (PE) + adds (DVE)
                        drain(sm1, 1)
                        more = drain(m0, 1)
                    if more:
                        drain(m0)             # PE: leftover mm2 fills the gap
                    emit_attT(s1)
                    drain(gen_mm2(s1, split_last=3))

    nc.compile()
    return nc


def _get_module():
    if "nc" not in _CACHE:
        _CACHE["nc"] = _build()
    return _CACHE["nc"]


def prepare_in_maps(x_high, x_low, gamma):
    x_high = np.asarray(x_high)
    x_low = np.asarray(x_low)
    gamma = np.asarray(gamma, dtype=np.float32).reshape(-1)

    xh3 = x_high.reshape(B, C, N).astype(np.float16)
    xl3 = x_low.reshape(B, C, N).astype(np.float16)
    xh3T = np.ascontiguousarray(xh3.transpose(0, 2, 1))
    gm = np.full((P, 1), gamma[0], dtype=np.float32)

    in_maps = []
    for i in range(N_CORES):
        sl = slice(i * B_LOC, (i + 1) * B_LOC)
        in_maps.append({
            "xh": np.ascontiguousarray(xh3[sl]),
            "xhT": np.ascontiguousarray(xh3T[sl]),
            "xl": np.ascontiguousarray(xl3[sl]),
            "gm": gm,
        })
    return in_maps


def kernel(x_high, x_low, gamma):
    from concourse.bass_utils import run_bass_kernel_spmd

    nc = _get_module()
    in_maps = prepare_in_maps(x_high, x_low, gamma)
    res = run_bass_kernel_spmd(nc, in_maps, list(range(N_CORES)))
    out = np.concatenate([res.results[i]["out"] for i in range(N_CORES)], axis=0)
    return out.reshape(B, C, H, W)
